# revision 2
# baseline (speedup 1.0000x reference)
"""Trainium2 Bass kernel for nn_CausalSelfAttention (BitNet-style GQA block).

Strategy (8 NeuronCores): 2-way data parallel over batch x 4-way tensor
parallel over kv-heads.  Core c = (b, h) with b = c // 4, h = c % 4 computes:
  - k, v projections for kv-head h (all 2048 positions)
  - q projections for q-heads 4h..4h+3
  - causal GQA attention for those 4 q-heads
  - transposed attention output yT for its 512 channels (+ partial sum-of-
    squares row for the final RMS norm), AllGather within the batch group
  - final projection against its 512-column shard of w_proj; the RMS scale
    is applied to the projection output (valid since the norm is a per-row
    scalar and the projection is linear)
Host assembles out[b, :, h*512:(h+1)*512] from each core.  Weights are
ternary-quantized on the host exactly as the reference does (bf16 values);
device matmuls run in bf16 with f32 accumulation.
"""

import math

import numpy as np
import ml_dtypes

B = 2
S = 2048
D = 2048
P = 128
NCC = D // P   # contraction chunks
NSC = S // P   # sequence chunks
HQ = 4         # q heads per core
HD = 128       # head dim
EPS = 1.1920929e-07
NCORES = 8
ROPE_BASE = 10000.0

_cache = {}


def _build_nc():
    import concourse.mybir as mybir
    import concourse.tile as tile
    from concourse import bacc
    from concourse.masks import make_identity

    bf16, f32 = mybir.dt.bfloat16, mybir.dt.float32
    AF = mybir.ActivationFunctionType
    ALU = mybir.AluOpType

    nc = bacc.Bacc("TRN2", num_devices=NCORES)

    xT_d = nc.dram_tensor("xT", [D, S], bf16, kind="ExternalInput")
    wq_d = nc.dram_tensor("wq", [D, HQ * HD], bf16, kind="ExternalInput")
    wkv_d = nc.dram_tensor("wkv", [D, 2 * HD], bf16, kind="ExternalInput")
    wp_d = nc.dram_tensor("wp", [D, 512], bf16, kind="ExternalInput")
    cos_d = nc.dram_tensor("cosb", [P, NSC, 64], f32, kind="ExternalInput")
    sin_d = nc.dram_tensor("sinb", [P, NSC, 64], f32, kind="ExternalInput")
    gain_d = nc.dram_tensor("gain", [P, HQ], f32, kind="ExternalInput")
    mask_d = nc.dram_tensor("maskT", [P, P], f32, kind="ExternalInput")
    out_d = nc.dram_tensor("out", [S, 512], f32, kind="ExternalOutput")
    cc_in = nc.dram_tensor("cc_in", [513, S], bf16, kind="Internal")
    cc_out = nc.dram_tensor("cc_out", [4, 513, S], bf16, kind="Internal")

    with tile.TileContext(nc) as tc:
        with (
            tc.tile_pool(name="const", bufs=1) as cp,
            tc.tile_pool(name="tmp", bufs=4) as tp,
        ):
            cos_sb = cp.tile([P, NSC, 64], f32)
            nc.sync.dma_start(cos_sb[:], cos_d[:])
            sin_sb = cp.tile([P, NSC, 64], f32)
            nc.sync.dma_start(sin_sb[:], sin_d[:])
            gain_sb = cp.tile([P, HQ], f32)
            nc.sync.dma_start(gain_sb[:], gain_d[:])
            mask_sb = cp.tile([P, P], f32)
            nc.sync.dma_start(mask_sb[:], mask_d[:])
            eps_sb = cp.tile([P, 1], f32)
            nc.vector.memset(eps_sb[:], EPS)
            ident = cp.tile([P, P], bf16)
            make_identity(nc, ident[:])

            wq_sb = cp.tile([P, NCC, HQ * HD], bf16)
            wkv_sb = cp.tile([P, NCC, 2 * HD], bf16)
            for cc in range(NCC):
                nc.sync.dma_start(wq_sb[:, cc, :], wq_d[cc * P : (cc + 1) * P, :])
                nc.sync.dma_start(wkv_sb[:, cc, :], wkv_d[cc * P : (cc + 1) * P, :])

            kT = cp.tile([P, NSC, P], bf16)
            v_sb = cp.tile([P, NSC, HD + 1], bf16)
            nc.vector.memset(v_sb[:, :, HD : HD + 1], 1.0)
            qT = cp.tile([P, HQ, NSC, P], bf16)
            y_sb = cp.tile([P, NSC, HQ * HD], bf16)
            yT_sb = cp.tile([P, HQ, S], bf16)
            ssqy = cp.tile([P, NSC], f32)
            ssqy_bf = cp.tile([P, NSC], bf16)

            def rms_rope(ps3, nh, sc, dst3, gain):
                """ps3: [P, nh, HD] psum f32; dst3: [P, nh, HD] sbuf bf16.

                dst = rope(ps3) * rsqrt(mean(ps3^2, -1) + eps) [* gain]
                """
                scr = tp.tile([P, nh, HD], f32, tag=f"rr_scr{nh}")
                ssq = tp.tile([P, nh], f32, tag=f"rr_ssq{nh}")
                for h in range(nh):
                    nc.scalar.activation(
                        scr[:, h], ps3[:, h], AF.Square,
                        accum_out=ssq[:, h : h + 1],
                    )
                rt = tp.tile([P, nh], f32, tag=f"rr_rt{nh}")
                nc.scalar.activation(
                    rt[:], ssq[:], AF.Sqrt, bias=eps_sb[:], scale=1.0 / HD
                )
                rr = tp.tile([P, nh], f32, tag=f"rr_r{nh}")
                nc.vector.reciprocal(rr[:], rt[:])
                if gain is not None:
                    nc.vector.tensor_mul(rr[:], rr[:], gain[:, :nh])
                cs = cos_sb[:, sc]
                sn = sin_sb[:, sc]
                cosb = cs[:, None, :].to_broadcast((P, nh, 64))
                sinb = sn[:, None, :].to_broadcast((P, nh, 64))
                rb = rr[:, :, None].to_broadcast((P, nh, 64))
                x1 = ps3[:, :, :64]
                x2 = ps3[:, :, 64:]
                t1 = tp.tile([P, nh, 64], f32, tag=f"rr_t1{nh}")
                t2 = tp.tile([P, nh, 64], f32, tag=f"rr_t2{nh}")
                nc.vector.tensor_mul(t1[:], x1, cosb)
                nc.vector.tensor_mul(t2[:], x2, sinb)
                nc.vector.tensor_add(t1[:], t1[:], t2[:])
                nc.vector.tensor_mul(dst3[:, :, :64], t1[:], rb)
                nc.vector.tensor_mul(t1[:], x2, cosb)
                nc.vector.tensor_mul(t2[:], x1, sinb)
                nc.vector.tensor_tensor(t1[:], t1[:], t2[:], ALU.subtract)
                nc.vector.tensor_mul(dst3[:, :, 64:], t1[:], rb)

            # ---- phase A: qkv projections + norm/rope + transposes ----
            with (
                tc.tile_pool(name="xt", bufs=1) as xp,
                tc.tile_pool(name="ps_a", bufs=2, space="PSUM") as pa,
                tc.tile_pool(name="ps_t", bufs=2, space="PSUM") as pt_ps,
            ):
                xt_sb = xp.tile([P, NCC, S], bf16)
                for cc in range(NCC):
                    nc.sync.dma_start(
                        xt_sb[:, cc, :], xT_d[cc * P : (cc + 1) * P, :]
                    )

                for sc in range(NSC):
                    ps = pa.tile([P, 2 * HD], f32, tag="kv")
                    for cc in range(NCC):
                        nc.tensor.matmul(
                            ps[:],
                            xt_sb[:, cc, sc * P : (sc + 1) * P],
                            wkv_sb[:, cc, :],
                            start=(cc == 0),
                            stop=(cc == NCC - 1),
                        )
                    kb = tp.tile([P, 1, HD], bf16, tag="kb")
                    rms_rope(
                        ps[:, :HD].rearrange("p (o d) -> p o d", o=1),
                        1, sc, kb, None,
                    )
                    pst = pt_ps.tile([P, P], bf16, tag="tp")
                    nc.tensor.transpose(pst[:], kb[:, 0], ident[:])
                    nc.vector.tensor_copy(out=kT[:, sc, :], in_=pst[:])
                    nc.vector.tensor_copy(
                        out=v_sb[:, sc, :HD], in_=ps[:, HD : 2 * HD]
                    )

                for sc in range(NSC):
                    ps = pa.tile([P, HQ * HD], f32, tag="q")
                    for cc in range(NCC):
                        nc.tensor.matmul(
                            ps[:],
                            xt_sb[:, cc, sc * P : (sc + 1) * P],
                            wq_sb[:, cc, :],
                            start=(cc == 0),
                            stop=(cc == NCC - 1),
                        )
                    qb = tp.tile([P, HQ, HD], bf16, tag="qb")
                    rms_rope(
                        ps.rearrange("p (h d) -> p h d", h=HQ),
                        HQ, sc, qb, gain_sb,
                    )
                    for h in range(HQ):
                        pst = pt_ps.tile([P, P], bf16, tag="tp")
                        nc.tensor.transpose(pst[:], qb[:, h], ident[:])
                        nc.vector.tensor_copy(out=qT[:, h, sc, :], in_=pst[:])

            # ---- phase B: causal attention ----
            with (
                tc.tile_pool(name="ptp", bufs=2) as ptp,
                tc.tile_pool(name="ps_st", bufs=3, space="PSUM") as pst_p,
                tc.tile_pool(name="ps_y", bufs=3, space="PSUM") as py_p,
                tc.tile_pool(name="ps_t2", bufs=2, space="PSUM") as pt2_p,
            ):
                for h in range(HQ):
                    for b4 in range(4):
                        # scores transposed: ST[sk, sq] for the 4 sq-chunks
                        # of block b4, exp'ed into ptb rows (one per sk-chunk)
                        ptb = ptp.tile([P, NSC, 512], bf16, tag="pt")
                        for c in range(4 * b4 + 4):
                            st = pst_p.tile([P, 512], f32, tag="st")
                            nc.tensor.matmul(
                                st[:],
                                kT[:, c, :],
                                qT[:, h, 4 * b4 : 4 * b4 + 4, :],
                                start=True,
                                stop=True,
                            )
                            if c >= 4 * b4:
                                off = (c - 4 * b4) * P
                                nc.vector.tensor_add(
                                    st[:, off : off + P],
                                    st[:, off : off + P],
                                    mask_sb[:],
                                )
                            nc.scalar.activation(ptb[:, c, :], st[:], AF.Exp)
                        for i in range(4):
                            a = 4 * b4 + i
                            yp = py_p.tile([P, HD + 1], f32, tag="y")
                            for c in range(a + 1):
                                nc.tensor.matmul(
                                    yp[:],
                                    ptb[:, c, i * P : (i + 1) * P],
                                    v_sb[:, c, :],
                                    start=(c == 0),
                                    stop=(c == a),
                                )
                            dnr = tp.tile([P, 1], f32, tag="dnr")
                            nc.vector.reciprocal(dnr[:], yp[:, HD : HD + 1])
                            nc.vector.tensor_scalar_mul(
                                y_sb[:, a, h * HD : (h + 1) * HD],
                                yp[:, :HD],
                                dnr[:],
                            )

                # partial sum-of-squares (for final RMS) + transpose y
                for a in range(NSC):
                    scr2 = tp.tile([P, HQ * HD], f32, tag="yscr")
                    nc.scalar.activation(
                        scr2[:], y_sb[:, a, :], AF.Square,
                        accum_out=ssqy[:, a : a + 1],
                    )
                    for h in range(HQ):
                        pst = pt2_p.tile([P, P], bf16, tag="t2")
                        nc.tensor.transpose(
                            pst[:], y_sb[:, a, h * HD : (h + 1) * HD], ident[:]
                        )
                        nc.vector.tensor_copy(
                            out=yT_sb[:, h, a * P : (a + 1) * P], in_=pst[:]
                        )
                nc.vector.tensor_copy(out=ssqy_bf[:], in_=ssqy[:])

                # ---- AllGather y (transposed) + ssq partials ----
                nc.sync.dma_start(
                    cc_in[0:512, :].rearrange("(h p) s -> p h s", p=P), yT_sb[:]
                )
                nc.sync.dma_start(
                    cc_in[512, :].rearrange("(a p) -> p a", p=P), ssqy_bf[:]
                )
                nc.gpsimd.collective_compute(
                    "AllGather",
                    ALU.bypass,
                    replica_groups=[[0, 1, 2, 3], [4, 5, 6, 7]],
                    ins=[cc_in[:]],
                    outs=[cc_out[:]],
                )

            # ---- phase C: final RMS-scaled projection ----
            with (
                tc.tile_pool(name="wp", bufs=1) as wpp,
                tc.tile_pool(name="pj", bufs=2) as pj,
                tc.tile_pool(name="ps_o", bufs=2, space="PSUM") as po_p,
            ):
                wp_sb = wpp.tile([P, NCC, 512], bf16)
                for cc in range(NCC):
                    nc.sync.dma_start(wp_sb[:, cc, :], wp_d[cc * P : (cc + 1) * P, :])
                ssqp = wpp.tile([P, NSC, 4], bf16)
                for r_ in range(4):
                    nc.sync.dma_start(
                        ssqp[:, :, r_],
                        cc_out[r_, 512, :].rearrange("(a p) -> p a", p=P),
                    )
                ssqt = wpp.tile([P, NSC], f32)
                nc.vector.tensor_reduce(
                    ssqt[:], ssqp[:], axis=mybir.AxisListType.X, op=ALU.add
                )
                rt2 = wpp.tile([P, NSC], f32)
                nc.scalar.activation(
                    rt2[:], ssqt[:], AF.Sqrt, bias=eps_sb[:], scale=1.0 / D
                )
                r2 = wpp.tile([P, NSC], f32)
                nc.vector.reciprocal(r2[:], rt2[:])

                for b4 in range(4):
                    ynt = pj.tile([P, NCC, 512], bf16, tag="ynt")
                    for r_ in range(4):
                        for hh in range(4):
                            nc.sync.dma_start(
                                ynt[:, r_ * 4 + hh, :],
                                cc_out[r_, hh * P : (hh + 1) * P,
                                       b4 * 512 : (b4 + 1) * 512],
                            )
                    for i in range(4):
                        a = b4 * 4 + i
                        po = po_p.tile([P, 512], f32, tag="o")
                        for cc in range(NCC):
                            nc.tensor.matmul(
                                po[:],
                                ynt[:, cc, i * P : (i + 1) * P],
                                wp_sb[:, cc, :],
                                start=(cc == 0),
                                stop=(cc == NCC - 1),
                            )
                        ob = pj.tile([P, 512], f32, tag="ob")
                        nc.vector.tensor_scalar_mul(ob[:], po[:], r2[:, a : a + 1])
                        nc.sync.dma_start(out_d[a * P : (a + 1) * P, :], ob[:])

    nc.compile()
    return nc


def _ternary_bf16(w):
    """Exact replica of the reference TernaryLinear weight path (bf16)."""
    import jax
    import jax.numpy as jnp

    cpu = jax.devices("cpu")[0]
    with jax.default_device(cpu):
        wb = jnp.asarray(np.asarray(w)).astype(jnp.bfloat16)
        wg = wb.reshape(-1, 128)
        scale = jnp.clip(jnp.mean(jnp.abs(wg), axis=-1, keepdims=True), 1e-8, None)
        q = jnp.clip(jnp.round(wg / scale), -1.0, 1.0)
        wt = wb + ((q * scale).reshape(wb.shape) - wb)
        return np.asarray(wt)


def _rope_tables():
    inv_freq = (1.0 / (np.float32(ROPE_BASE) ** (
        np.arange(0, HD, 2, dtype=np.float32) / np.float32(HD)))).astype(np.float32)
    t = np.arange(S, dtype=np.float32)
    freqs = np.outer(t, inv_freq).astype(np.float32)  # [S, 64]
    cos = np.cos(freqs).astype(np.float32)
    sin = np.sin(freqs).astype(np.float32)
    # [S, 64] -> [P, NSC, 64] with s = chunk*128 + p
    cos_sb = np.ascontiguousarray(cos.reshape(NSC, P, 64).transpose(1, 0, 2))
    sin_sb = np.ascontiguousarray(sin.reshape(NSC, P, 64).transpose(1, 0, 2))
    return cos_sb, sin_sb


def kernel(x, w_qkv, w_proj, q_gain):
    from concourse.bass_utils import run_bass_kernel_spmd

    bf = ml_dtypes.bfloat16
    x = np.asarray(x, dtype=np.float32)
    w_qkv = np.asarray(w_qkv, dtype=np.float32)
    w_proj = np.asarray(w_proj, dtype=np.float32)
    q_gain = np.asarray(q_gain, dtype=np.float32)

    wt_qkv = _ternary_bf16(w_qkv)   # [3072, 2048] bf16
    wt_proj = _ternary_bf16(w_proj)  # [2048, 2048] bf16
    cos_sb, sin_sb = _rope_tables()
    maskT = np.where(
        np.arange(P)[:, None] <= np.arange(P)[None, :], 0.0, -1e30
    ).astype(np.float32)

    xT = [np.ascontiguousarray(x[b].T.astype(bf)) for b in range(B)]
    scale = np.float32(1.0) / np.sqrt(np.float32(HD))

    in_maps = []
    for core in range(NCORES):
        b, h = divmod(core, 4)
        wq = np.ascontiguousarray(wt_qkv[h * 512 : (h + 1) * 512, :].T)
        wkv = np.ascontiguousarray(
            np.concatenate(
                [
                    wt_qkv[2048 + h * P : 2048 + (h + 1) * P, :],
                    wt_qkv[2560 + h * P : 2560 + (h + 1) * P, :],
                ],
                axis=0,
            ).T
        )
        wp = np.ascontiguousarray(wt_proj[h * 512 : (h + 1) * 512, :].T)
        gain = np.ascontiguousarray(
            np.broadcast_to(
                (q_gain[4 * h : 4 * h + 4] * scale).astype(np.float32), (P, HQ)
            )
        )
        in_maps.append(
            {
                "xT": xT[b],
                "wq": wq,
                "wkv": wkv,
                "wp": wp,
                "cosb": cos_sb,
                "sinb": sin_sb,
                "gain": gain,
                "maskT": maskT,
            }
        )

    global _last_in_maps
    _last_in_maps = in_maps

    if "nc" not in _cache:
        _cache["nc"] = _build_nc()
    nc = _cache["nc"]

    res = run_bass_kernel_spmd(nc, in_maps, core_ids=list(range(NCORES)))

    out = np.empty((B, S, D), dtype=np.float32)
    for core in range(NCORES):
        b, h = divmod(core, 4)
        out[b, :, h * 512 : (h + 1) * 512] = res.results[core]["out"]
    return out
